# revision 24
# baseline (speedup 1.0000x reference)
"""Trainium2 Bass kernel for nn_DFTModel (segment_reduce).

Math: for each level l in {1,2,3} the reference multiplies channel l-1 by a
block-tiled radial ("square ring") filter and sums nested square rectangles.
Nested-square sums of the raw image Q[r] are computed as
    Q = rowsum(M (*) (A~ @ X))     (PE matmul with 0/1 interval operator A~,
                                    DVE mask-multiply + reduce, with u1/u2
                                    reductions offloaded to the scalar engine)
and the filter weights fold into a tiny per-block upper-triangular transform
    c = U @ Q,   U[r,r] = wv[r], U[r,d>r] = wv[d] - wv[d-1]
so the 222x222 filter product is never materialized.  Phase 2 places each
block's c-slice directly at its final feature partition via a host-built
[128, 668] operator (one matmul per 128-boundary segment, no scatter DMAs).

DMA layout: every HBM->SBUF transfer uses exactly 128 partitions (transfers
with fewer partitions land on only 2 of the 16 SDMA engines).  x is re-laid
on the host to [group, half, 128, img, 1026] bf16: half 0 = image rows
0..127, half 1 = rows 94..221 (34-row overlap killed by zero rows in the
interval operator), elems padded 666 -> 1026 = 342*3 so stride-3 channel
slicing is preserved.  MLP weights are bf16 with k-tiles padded to multiples
of 128 rows and the bias row folded into the last k-tile (matching a ones
row in the activations).

Sharding: pure data parallel, 32 images per core on 8 cores.  BatchNorm
batch statistics are exchanged with one 6 KB AllReduce; the MLP weights
stream from HBM underneath that collective.
"""

import os
import sys
import types

import numpy as np
import ml_dtypes

for _p in ("/opt/trn_rl_repo",):
    if _p not in sys.path and os.path.isdir(_p):
        sys.path.insert(0, _p)

import concourse.bacc as bacc
import concourse.bass as bass
import concourse.mybir as mybir
import concourse.tile as tile

F32 = mybir.dt.float32
BF16 = mybir.dt.bfloat16
BF16NP = ml_dtypes.bfloat16

IMAGE = 222
EPAD = 1026            # padded elems per image row (342 * 3)
N_CORES = 8
IMG_PER_CORE = 32
NUM_CLASSES = 1000
FC_DIMS = [668, 2048, 1024, 512, 128, 1000]
H1 = 128               # rows in half 0
H2_OFF = 94            # half 1 = rows 94..221 (34-row overlap with half 0)


def _get_dim(level):
    d = IMAGE / level
    if d % 2 == 1:
        d += 1
    return int(d // 2)


MAXR = {l: _get_dim(l) for l in (1, 2, 3)}          # 111, 56, 37
FBLK = {l: IMAGE // l for l in (1, 2, 3)}           # 222, 111, 74
NUM_COEFF = sum(l * l * MAXR[l] for l in (1, 2, 3))  # 668

# step-1 units: (name, level, [(bi, col_offset)...], n_rows)
UNITS = [
    ("u1", 1, [(0, 0)], 111),
    ("u2", 2, [(0, 0), (1, 64)], 120),
    ("u3", 3, [(0, 0), (1, 37), (2, 74)], 111),
]

# units whose reduction runs on the scalar engine (per (img, bj) slices)
SCALAR_REDUCE = {"u1"}


def _block_offsets():
    offs = {}
    base = 0
    for l in (1, 2, 3):
        for bi in range(l):
            for bj in range(l):
                offs[(l, bi, bj)] = base + (bi * l + bj) * MAXR[l]
        base += l * l * MAXR[l]
    return offs


BLOCK_OFF = _block_offsets()


# ---------------------------------------------------------------- host consts
def _full_A(level, bi, n_cols, col_off):
    """interval operator rows for block-row bi of `level`: A[row, col_off+r]=1
    for row in [bi*F+r, (bi+1)*F-r)."""
    F, R = FBLK[level], MAXR[level]
    A = np.zeros((IMAGE, n_cols), np.float32)
    for r in range(R):
        A[bi * F + r:(bi + 1) * F - r, col_off + r] = 1.0
    return A


def _unit_A2(name):
    """[128, 2, n] operator: slot 0 = image rows 0..127, slot 1 = rows
    94..221 with the first 34 rows zeroed (already counted in slot 0)."""
    for nm, lv, pk, nr in UNITS:
        if nm == name:
            l, packs, n = lv, pk, nr
    A = np.zeros((IMAGE, n), np.float32)
    for bi, off in packs:
        A += _full_A(l, bi, n, off)
    out = np.zeros((128, 2, n), np.float32)
    out[:, 0, :] = A[0:128]
    out[34:128, 1, :] = A[128:222]
    return np.ascontiguousarray(out.astype(BF16NP))


def _unit_M(name):
    """Mask replicated over (2 imgs, l block-cols): (128, 2, l, F) f32,
    zero on pad rows."""
    for nm, lv, pk, nr in UNITS:
        if nm == name:
            l, packs, n = lv, pk, nr
    F, R = FBLK[l], MAXR[l]
    M = np.zeros((128, F), np.float32)
    for _, off in packs:
        for r in range(R):
            M[off + r, r:F - r] = 1.0
    return np.ascontiguousarray(
        np.broadcast_to(M[:, None, None, :], (128, 2, l, F))).reshape(128, -1)


def _phase2_windows():
    """One matmul per (block, k-tile) overlap: each writes a full 128-wide
    PSUM k-tile (base partition 0) through a dedicated 128-col UT window.
    q source key: (nm, bi); u3 blocks bi>=1 read re-aligned base-0 q copies.
    Returns [(nm, bi, qoff, R, bj, f0, kt, w)] sorted by kt."""
    unit_of = {}
    for nm, l, pk, n in UNITS:
        for bi, off in pk:
            unit_of[(l, bi)] = (nm, off)
    wins = []
    for l in (1, 2, 3):
        R = MAXR[l]
        for bi in range(l):
            nm, off = unit_of[(l, bi)]
            # rhs base partition must be 0, 32, or 64
            qoff = off if off in (0, 32, 64) else 0
            for bj in range(l):
                f0 = BLOCK_OFF[(l, bi, bj)]
                for kt in range(f0 // 128, (f0 + R - 1) // 128 + 1):
                    wins.append((nm, bi, qoff, R, bj, f0, kt))
    wins.sort(key=lambda t: t[6])
    return [(nm, bi, qoff, R, bj, f0, kt, w)
            for w, (nm, bi, qoff, R, bj, f0, kt) in enumerate(wins)]


P2WINS = _phase2_windows()
NP2 = len(P2WINS)


def _build_UTP(ws):
    """[128, NP2, 128] f32 window operator: window w (for block (nm,bj) x
    k-tile kt) holds U^T entries at rows off..off+R, col p = f0 + d - kt*128
    for the in-window coefficient range."""
    out = np.zeros((128, NP2, 128), np.float32)
    for (nm, bi, qoff, R, bj, f0, kt, w) in P2WINS:
        for nm2, l, pk, n in UNITS:
            if nm2 == nm:
                F = FBLK[l]
                break
        wv = np.array([ws[l][bi * l + bj][(F - 1) // 2 - d]
                       for d in range(R)], np.float32)
        U = np.zeros((R, R), np.float32)
        for r in range(R):
            U[r, r] = wv[r]
            U[r, r + 1:] = wv[r + 1:] - wv[r:-1]
        d0 = max(0, kt * 128 - f0)
        d1 = min(R, (kt + 1) * 128 - f0)
        p0 = f0 + d0 - kt * 128
        out[qoff:qoff + R, w, p0:p0 + (d1 - d0)] = U.T[:, d0:d1]
    return out.reshape(128, -1)


def _feat_major(v, ktiles):
    out = np.zeros((128, ktiles), np.float32)
    n = v.shape[0]
    for kt in range(ktiles):
        lo, hi = kt * 128, min((kt + 1) * 128, n)
        if lo >= n:
            break
        out[:hi - lo, kt] = v[lo:hi]
    return out


# ---------------------------------------------------------------- bass build
def build_program(n_cores=N_CORES, img_per_core=IMG_PER_CORE):
    B = img_per_core
    NG = B // 2
    assert B % 2 == 0
    nc = bacc.Bacc("TRN2", target_bir_lowering=False, debug=False,
                   num_devices=n_cores)

    AX = mybir.AxisListType
    OP = mybir.AluOpType
    ACT = mybir.ActivationFunctionType

    # ---- DRAM I/O
    # x re-laid on host: [group, half, 128, img-in-pair, EPAD] bf16
    x = nc.dram_tensor("x", [NG, 128, 2, 2, EPAD], BF16, kind="ExternalInput")
    at_d = {nm: nc.dram_tensor(f"at_{nm}", [128, 2 * n], BF16,
                               kind="ExternalInput")
            for nm, l, pk, n in UNITS}
    mk_d = {nm: nc.dram_tensor(f"mk_{nm}", [128, 2 * IMAGE], F32,
                               kind="ExternalInput")
            for nm, l, pk, n in UNITS}
    ut_d = nc.dram_tensor("ut", [128, NP2 * 128], F32, kind="ExternalInput")
    wt_d, WKT = {}, {}
    for i in range(1, 6):
        kin, nout = FC_DIMS[i - 1], FC_DIMS[i]
        WKT[i] = (kin + 1 + 127) // 128
        wt_d[i] = nc.dram_tensor(f"w{i}t", [WKT[i] * 128, nout], BF16,
                                 kind="ExternalInput")
    gm_d = nc.dram_tensor("gmt", [128, 6], F32, kind="ExternalInput")
    be_d = nc.dram_tensor("bet", [128, 6], F32, kind="ExternalInput")
    id_d = nc.dram_tensor("idn", [B, B], BF16, kind="ExternalInput")
    on_d = nc.dram_tensor("ones", [1, B], BF16, kind="ExternalInput")
    out_d = nc.dram_tensor("out", [B, NUM_CLASSES], F32, kind="ExternalOutput")

    with tile.TileContext(nc) as tc:
        with tc.tile_pool(name="consts", bufs=1) as cp_pool, \
             tc.tile_pool(name="xg", bufs=8) as xg_pool, \
             tc.tile_pool(name="scr", bufs=4) as scr_pool, \
             tc.tile_pool(name="wc", bufs=6) as wc_pool, \
             tc.tile_pool(name="dram", bufs=1, space="DRAM") as dram_pool:

            # ---- constants into SBUF (sync ring; x streams on scalar ring)
            at_sb, mk_sb, q_sb = {}, {}, {}
            for nm, l, pk, n in UNITS:
                at_sb[nm] = cp_pool.tile([128, 2, n], BF16, name=f"at_{nm}_sb")
                nc.sync.dma_start(out=at_sb[nm][:], in_=at_d[nm].ap())
                mk_sb[nm] = cp_pool.tile([128, 2, l, FBLK[l]], F32,
                                         name=f"mk_{nm}_sb")
                nc.sync.dma_start(out=mk_sb[nm][:], in_=mk_d[nm].ap())
                q_sb[(nm, 0)] = cp_pool.tile([n, B // 2, l], F32,
                                             name=f"q_{nm}a")
                q_sb[(nm, 1)] = cp_pool.tile([n, B // 2, l], F32,
                                             name=f"q_{nm}b")
            ut_sb = cp_pool.tile([128, NP2, 128], F32, name="ut_sb")
            nc.sync.dma_start(out=ut_sb[:], in_=ut_d.ap())
            gm_sb = cp_pool.tile([128, 6], F32, name="gm_sb")
            be_sb = cp_pool.tile([128, 6], F32, name="be_sb")
            id_sb = cp_pool.tile([B, B], BF16, name="id_sb")
            nc.sync.dma_start(out=gm_sb[:], in_=gm_d.ap())
            nc.sync.dma_start(out=be_sb[:], in_=be_d.ap())
            nc.sync.dma_start(out=id_sb[:], in_=id_d.ap())
            junk = cp_pool.tile([128, IMAGE], BF16, name="junk")

            # ---- phase 2+3 per 16-image half: c = U @ Q at final feature
            # partitions, then BN partial stats + AllReduce
            HB = B // 2
            c_sb = {h: cp_pool.tile([128, 6, HB], F32, name=f"c_sb{h}")
                    for h in (0, 1)}
            csq = {h: cp_pool.tile([128, 6, HB], F32, name=f"csq{h}")
                   for h in (0, 1)}
            stats = {h: cp_pool.tile([128, 12], F32, name=f"stats{h}")
                     for h in (0, 1)}
            statg = {h: cp_pool.tile([128, 12], F32, name=f"statg{h}")
                     for h in (0, 1)}
            cc_in = {h: dram_pool.tile([128, 12], F32, name=f"cc_in{h}")
                     for h in (0, 1)}
            cc_out = {h: dram_pool.tile(
                [128, 12], F32, name=f"cc_out{h}",
                addr_space="Shared" if n_cores > 4 else "Local")
                for h in (0, 1)}
            kt_first = {}
            for (nm, bi, qoff, R, bj, f0, kt, w) in P2WINS:
                kt_first.setdefault(kt, w)

            def _phase2_stats(h):
                cs = c_sb[h]
                qre = {}
                for bi, off in [(1, 37), (2, 74)]:
                    qre[bi] = cp_pool.tile([37, HB, 3], F32,
                                           name=f"qre{bi}h{h}")
                    nc.sync.dma_start(
                        out=qre[bi][:],
                        in_=q_sb[("u3", h)][off:off + 37, :, :])
                with tc.tile_pool(name=f"cps{h}", bufs=2,
                                  space="PSUM") as cps_pool:
                    for (nm, bi, qoff, R, bj, f0, kt, w) in P2WINS:
                        if nm == "u3" and bi >= 1:
                            qsrc = qre[bi][0:R, :, bj]
                        else:
                            qsrc = q_sb[(nm, h)][qoff:qoff + R, :, bj]
                        pw = cps_pool.tile([128, HB], F32, tag="pw",
                                           name=f"pw{w}h{h}")
                        nc.tensor.matmul(
                            pw[:], ut_sb[qoff:qoff + R, w, :], qsrc,
                            start=True, stop=True)
                        if kt_first[kt] == w:
                            nc.scalar.copy(cs[0:128, kt, :], pw[:])
                        else:
                            nc.vector.tensor_tensor(
                                cs[0:128, kt, :], cs[0:128, kt, :], pw[:],
                                OP.add)
                nc.vector.tensor_reduce(stats[h][0:128, 0:6], cs[:], AX.X,
                                        OP.add)
                nc.vector.tensor_tensor(csq[h][:], cs[:], cs[:], OP.mult)
                nc.vector.tensor_reduce(stats[h][0:128, 6:12], csq[h][:],
                                        AX.X, OP.add)
                nc.sync.dma_start(out=cc_in[h][:], in_=stats[h][:])
                nc.gpsimd.collective_compute(
                    "AllReduce", OP.add,
                    replica_groups=[list(range(n_cores))],
                    ins=[cc_in[h][:].opt()], outs=[cc_out[h][:].opt()])
                nc.sync.dma_start(out=statg[h][:], in_=cc_out[h][:])

            # ---- phase 1: stream images, segment-reduce to Q
            with tc.tile_pool(name="zp", bufs=2, space="PSUM") as zp_pool:
                for g in range(NG):
                    xg = xg_pool.tile([128, 2, 2, EPAD // 3, 3], BF16,
                                      tag="xg")
                    nc.scalar.dma_start(
                        out=xg[:], in_=x.ap()[g, 0:128, :, :, :])
                    for nm, l, pk, n in UNITS:
                        F = FBLK[l]
                        zp = zp_pool.tile([n, 2, l, F], F32, tag=f"z{nm}")
                        nc.tensor.matmul(
                            zp[:], at_sb[nm][0:128, 0, :],
                            xg[0:128, 0, :, 0:IMAGE, l - 1],
                            start=True, stop=False)
                        nc.tensor.matmul(
                            zp[:], at_sb[nm][0:128, 1, :],
                            xg[0:128, 1, :, 0:IMAGE, l - 1],
                            start=False, stop=True)
                        sc = scr_pool.tile([n, 2, l, F], BF16, tag=f"sc{nm}")
                        nc.vector.tensor_tensor(
                            sc[:], zp[:], mk_sb[nm][0:n, :, :, :], OP.mult)
                        hf_, gc = g // (NG // 2), g % (NG // 2)
                        if nm in SCALAR_REDUCE:
                            for j in (0, 1):
                                for bj in range(l):
                                    nc.scalar.activation(
                                        junk[0:n, 0:F], sc[0:n, j, bj, :],
                                        ACT.Copy,
                                        accum_out=q_sb[(nm, hf_)][
                                            0:n, 2 * gc + j, bj:bj + 1])
                        else:
                            nc.vector.tensor_reduce(
                                q_sb[(nm, hf_)][0:n, 2 * gc:2 * gc + 2, :],
                                sc[:], AX.X, OP.add)
                    if g == NG // 2 - 1:
                        _phase2_stats(0)
                _phase2_stats(1)

            # ---- phase 4: d = gamma * rsqrt(var + eps); c <- d*c + e
            nb = float(n_cores * B)
            statsum = cp_pool.tile([128, 12], F32, name="statsum")
            nc.vector.tensor_tensor(statsum[:], statg[0][:], statg[1][:],
                                    OP.add)
            bnd = cp_pool.tile([128, 6], F32, name="bnd")
            bne = cp_pool.tile([128, 6], F32, name="bne")
            mu = cp_pool.tile([128, 6], F32, name="mu")
            vtmp = cp_pool.tile([128, 6], F32, name="vtmp")
            nc.scalar.mul(mu[:], statsum[0:128, 0:6], 1.0 / nb)
            nc.scalar.mul(vtmp[:], statsum[0:128, 6:12], 1.0 / nb)
            nc.vector.tensor_tensor(bnd[:], mu[:], mu[:], OP.mult)
            nc.vector.tensor_tensor(vtmp[:], vtmp[:], bnd[:], OP.subtract)
            eps = cp_pool.tile([128, 1], F32, name="eps")
            nc.vector.memset(eps[:], 1e-5)
            nc.scalar.activation(vtmp[:], vtmp[:], ACT.Sqrt, bias=eps[:])
            nc.vector.reciprocal(vtmp[:], vtmp[:])
            nc.vector.tensor_tensor(bnd[:], gm_sb[:], vtmp[:], OP.mult)
            nc.vector.tensor_tensor(vtmp[:], mu[:], bnd[:], OP.mult)
            nc.vector.tensor_tensor(bne[:], be_sb[:], vtmp[:], OP.subtract)
            cn_sb = cp_pool.tile([128, 6, B], BF16, name="cn_sb")
            nc.vector.memset(cn_sb[0:128, 5, :], 0.0)
            # ones row for the fc1 bias at feature 668 = ktile 5, partition 28
            nc.sync.dma_start(out=cn_sb[28:29, 5, :], in_=on_d.ap())
            for kt in range(6):
                pmax = 128 if kt < 5 else 28
                for h in (0, 1):
                    nc.vector.tensor_scalar(
                        out=cn_sb[0:pmax, kt, h * HB:(h + 1) * HB],
                        in0=c_sb[h][0:pmax, kt, :],
                        scalar1=bnd[0:pmax, kt:kt + 1],
                        scalar2=bne[0:pmax, kt:kt + 1],
                        op0=OP.mult, op1=OP.add)

            # ---- phase 5: batch-major MLP, bf16 weights stream in chunks
            h_feat = cn_sb           # (128, KT, B) feature-major, bias ones in
            with tc.tile_pool(name="mps", bufs=1, space="PSUM") as mps_pool, \
                 tc.tile_pool(name="tps", bufs=2, space="PSUM") as tps_pool:
                for i in range(1, 6):
                    kin, nout = FC_DIMS[i - 1], FC_DIMS[i]
                    KT = WKT[i]
                    nslices = [(s, min(s + 512, nout))
                               for s in range(0, nout, 512)]
                    psums = [mps_pool.tile([B, n1 - n0], F32, tag=f"mp{si}",
                                           name=f"mp{i}_{si}")
                             for si, (n0, n1) in enumerate(nslices)]
                    for kt in range(KT):
                        wck = wc_pool.tile([128, nout], BF16, tag="wc")
                        nc.sync.dma_start(
                            out=wck[:],
                            in_=wt_d[i].ap()[128 * kt:128 * (kt + 1), :])
                        for si, (n0, n1) in enumerate(nslices):
                            nc.tensor.matmul(
                                psums[si][:],
                                h_feat[0:128, kt, :],
                                wck[0:128, n0:n1],
                                start=(kt == 0), stop=(kt == KT - 1))
                    if i == 5:
                        out_sb = cp_pool.tile([B, NUM_CLASSES], F32,
                                              name="out_sb")
                        for si, (n0, n1) in enumerate(nslices):
                            nc.scalar.copy(out_sb[0:B, n0:n1], psums[si][:])
                        break
                    h_b = cp_pool.tile([B, nout], BF16, name=f"hb{i}")
                    for si, (n0, n1) in enumerate(nslices):
                        nc.scalar.activation(h_b[0:B, n0:n1], psums[si][:],
                                             ACT.Relu)
                    # transpose back to feature-major; bias ones k-tile last
                    nkt = nout // 128
                    h_feat = cp_pool.tile([128, WKT[i + 1], B], BF16,
                                          name=f"hf{i}")
                    nc.vector.memset(h_feat[0:128, nkt, :], 0.0)
                    nc.sync.dma_start(out=h_feat[0:1, nkt, :],
                                      in_=on_d.ap())
                    for kt in range(nkt):
                        tp = tps_pool.tile([128, B], BF16, tag="tp")
                        nc.tensor.transpose(
                            tp[:], h_b[0:B, kt * 128:(kt + 1) * 128],
                            id_sb[:])
                        nc.scalar.copy(h_feat[0:128, kt, :], tp[:])
            nc.sync.dma_start(out=out_d.ap(), in_=out_sb[:])

    nc.compile()
    return nc


# ------------------------------------------------------------------- runtime
_CACHE = {}


def _get_program():
    key = (N_CORES, IMG_PER_CORE)
    if key not in _CACHE:
        _CACHE[key] = build_program(*key)
    return _CACHE[key]


def _host_consts(w1, w2, w3, bn_gamma, bn_beta, fcs):
    ws = {1: np.asarray(w1, np.float32).reshape(1, -1),
          2: np.asarray(w2, np.float32),
          3: np.asarray(w3, np.float32)}
    consts = {}
    for nm, l, pk, n in UNITS:
        consts[f"at_{nm}"] = _unit_A2(nm).reshape(128, -1)
        consts[f"mk_{nm}"] = _unit_M(nm)
    consts["ut"] = _build_UTP(ws)
    for i in range(1, 6):
        w, b = fcs[i - 1]
        kin, nout = FC_DIMS[i - 1], FC_DIMS[i]
        kt = (kin + 1 + 127) // 128
        wt = np.zeros((kt * 128, nout), np.float32)
        wt[0:kin] = np.asarray(w, np.float32).T
        wt[kin] = np.asarray(b, np.float32)
        consts[f"w{i}t"] = np.ascontiguousarray(wt.astype(BF16NP))
    consts["gmt"] = _feat_major(np.asarray(bn_gamma, np.float32), 6)
    consts["bet"] = _feat_major(np.asarray(bn_beta, np.float32), 6)
    consts["idn"] = np.eye(IMG_PER_CORE, dtype=BF16NP)
    consts["ones"] = np.ones((1, IMG_PER_CORE), BF16NP)
    return consts


def _layout_x(xs):
    """[32, 222, 222, 3] f32 -> [16, 128, 2, 2, EPAD] bf16: [group,
    partition, half, img-in-pair, elems] with half 0 = rows 0..127, half 1 =
    rows 94..221, elems padded 666 -> EPAD."""
    B = xs.shape[0]
    xi = xs.reshape(B, IMAGE, IMAGE * 3).astype(BF16NP)
    out = np.zeros((B // 2, 128, 2, 2, EPAD), BF16NP)
    # [B, rows, 666] -> groups of 2 imgs: [NG, 2, rows, 666]
    xg = xi.reshape(B // 2, 2, IMAGE, IMAGE * 3)
    out[:, :, 0, :, 0:IMAGE * 3] = xg[:, :, 0:128].transpose(0, 2, 1, 3)
    out[:, :, 1, :, 0:IMAGE * 3] = xg[:, :, H2_OFF:IMAGE].transpose(0, 2, 1, 3)
    return np.ascontiguousarray(out)


def kernel(x, w1, w2, w3, bn_gamma, bn_beta,
           fc1_w, fc1_b, fc2_w, fc2_b, fc3_w, fc3_b, fc4_w, fc4_b,
           fc5_w, fc5_b):
    from concourse.bass_utils import run_bass_kernel_spmd

    nc = _get_program()
    consts = _host_consts(
        w1, w2, w3, bn_gamma, bn_beta,
        [(fc1_w, fc1_b), (fc2_w, fc2_b), (fc3_w, fc3_b), (fc4_w, fc4_b),
         (fc5_w, fc5_b)])
    x = np.asarray(x, np.float32)
    in_maps = []
    for s in range(N_CORES):
        m = dict(consts)
        m["x"] = _layout_x(x[s * IMG_PER_CORE:(s + 1) * IMG_PER_CORE])
        in_maps.append(m)

    trace = bool(int(os.environ.get("BASSDFT_TRACE", "0")))
    if trace:
        _install_ntff_hook()
    res = run_bass_kernel_spmd(nc, in_maps, core_ids=list(range(N_CORES)),
                               trace=trace)
    if trace:
        kernel.last_exec_time_ns = res.exec_time_ns
        kernel.last_results = res
    return np.concatenate([res.results[s]["out"] for s in range(N_CORES)],
                          axis=0)


def _install_ntff_hook():
    """Register the axon NTFF profiling hook (antenv.axon_hooks is absent in
    this image) and disable the share-bucket artifact upload."""
    try:
        from antenv import axon_hooks  # noqa: F401
        return
    except ImportError:
        pass
    try:
        from trn_agent_boot.trn_boot import _ntff_profile_via_ctypes
    except ImportError:
        return
    import antenv
    import concourse.bass_utils as bu
    mod = types.ModuleType("antenv.axon_hooks")
    hook = [_ntff_profile_via_ctypes("/opt/axon/libaxon_pjrt.so")]
    mod.get_axon_ntff_profile_hook = lambda: hook[0]
    mod.set_axon_ntff_profile_hook = lambda h: hook.__setitem__(0, h)
    sys.modules["antenv.axon_hooks"] = mod
    antenv.axon_hooks = mod
    bu.upload_artifacts = lambda tmpdir: tmpdir


# revision 25
# speedup vs baseline: 1.1603x; 1.1603x over previous
"""Trainium2 Bass kernel for nn_DFTModel (segment_reduce).

Math: for each level l in {1,2,3} the reference multiplies channel l-1 by a
block-tiled radial ("square ring") filter and sums nested square rectangles.
Nested-square sums of the raw image Q[r] are computed as
    Q = rowsum(M (*) (A~ @ X))     (PE matmul with 0/1 interval operator A~,
                                    DVE mask-multiply + reduce, with u1/u2
                                    reductions offloaded to the scalar engine)
and the filter weights fold into a tiny per-block upper-triangular transform
    c = U @ Q,   U[r,r] = wv[r], U[r,d>r] = wv[d] - wv[d-1]
so the 222x222 filter product is never materialized.  Phase 2 places each
block's c-slice directly at its final feature partition via a host-built
[128, 668] operator (one matmul per 128-boundary segment, no scatter DMAs).

DMA layout: every HBM->SBUF transfer uses exactly 128 partitions (transfers
with fewer partitions land on only 2 of the 16 SDMA engines).  x is re-laid
on the host to [group, half, 128, img, 1026] bf16: half 0 = image rows
0..127, half 1 = rows 94..221 (34-row overlap killed by zero rows in the
interval operator), elems padded 666 -> 1026 = 342*3 so stride-3 channel
slicing is preserved.  MLP weights are bf16 with k-tiles padded to multiples
of 128 rows and the bias row folded into the last k-tile (matching a ones
row in the activations).

Sharding: pure data parallel, 32 images per core on 8 cores.  BatchNorm
batch statistics are exchanged with one 6 KB AllReduce; the MLP weights
stream from HBM underneath that collective.
"""

import os
import sys
import types

import numpy as np
import ml_dtypes

for _p in ("/opt/trn_rl_repo",):
    if _p not in sys.path and os.path.isdir(_p):
        sys.path.insert(0, _p)

import concourse.bacc as bacc
import concourse.bass as bass
import concourse.mybir as mybir
import concourse.tile as tile

F32 = mybir.dt.float32
BF16 = mybir.dt.bfloat16
BF16NP = ml_dtypes.bfloat16

IMAGE = 222
EPAD = 1026            # padded elems per image row (342 * 3)
N_CORES = 8
IMG_PER_CORE = 32
NUM_CLASSES = 1000
FC_DIMS = [668, 2048, 1024, 512, 128, 1000]
H1 = 128               # rows in half 0
H2_OFF = 94            # half 1 = rows 94..221 (34-row overlap with half 0)


def _get_dim(level):
    d = IMAGE / level
    if d % 2 == 1:
        d += 1
    return int(d // 2)


MAXR = {l: _get_dim(l) for l in (1, 2, 3)}          # 111, 56, 37
FBLK = {l: IMAGE // l for l in (1, 2, 3)}           # 222, 111, 74
NUM_COEFF = sum(l * l * MAXR[l] for l in (1, 2, 3))  # 668

# step-1 units: (name, level, [(bi, col_offset)...], n_rows)
UNITS = [
    ("u1", 1, [(0, 0)], 111),
    ("u2", 2, [(0, 0), (1, 64)], 120),
    ("u3", 3, [(0, 0), (1, 37), (2, 74)], 111),
]

# units whose reduction runs on the scalar engine (per (img, bj) slices)
SCALAR_REDUCE = {"u1"}


def _block_offsets():
    offs = {}
    base = 0
    for l in (1, 2, 3):
        for bi in range(l):
            for bj in range(l):
                offs[(l, bi, bj)] = base + (bi * l + bj) * MAXR[l]
        base += l * l * MAXR[l]
    return offs


BLOCK_OFF = _block_offsets()


# ---------------------------------------------------------------- host consts
def _full_A(level, bi, n_cols, col_off):
    """interval operator rows for block-row bi of `level`: A[row, col_off+r]=1
    for row in [bi*F+r, (bi+1)*F-r)."""
    F, R = FBLK[level], MAXR[level]
    A = np.zeros((IMAGE, n_cols), np.float32)
    for r in range(R):
        A[bi * F + r:(bi + 1) * F - r, col_off + r] = 1.0
    return A


def _unit_A2(name):
    """[128, 2, n] operator: slot 0 = image rows 0..127, slot 1 = rows
    94..221 with the first 34 rows zeroed (already counted in slot 0)."""
    for nm, lv, pk, nr in UNITS:
        if nm == name:
            l, packs, n = lv, pk, nr
    A = np.zeros((IMAGE, n), np.float32)
    for bi, off in packs:
        A += _full_A(l, bi, n, off)
    out = np.zeros((128, 2, n), np.float32)
    out[:, 0, :] = A[0:128]
    out[34:128, 1, :] = A[128:222]
    return np.ascontiguousarray(out.astype(BF16NP))


def _unit_M(name):
    """Mask replicated over (2 imgs, l block-cols): (128, 2, l, F) f32,
    zero on pad rows."""
    for nm, lv, pk, nr in UNITS:
        if nm == name:
            l, packs, n = lv, pk, nr
    F, R = FBLK[l], MAXR[l]
    M = np.zeros((128, F), np.float32)
    for _, off in packs:
        for r in range(R):
            M[off + r, r:F - r] = 1.0
    return np.ascontiguousarray(
        np.broadcast_to(M[:, None, None, :], (128, 2, l, F))).reshape(128, -1)


def _phase2_windows():
    """One matmul per (block, k-tile) overlap: each writes a full 128-wide
    PSUM k-tile (base partition 0) through a dedicated 128-col UT window.
    q source key: (nm, bi); u3 blocks bi>=1 read re-aligned base-0 q copies.
    Returns [(nm, bi, qoff, R, bj, f0, kt, w)] sorted by kt."""
    unit_of = {}
    for nm, l, pk, n in UNITS:
        for bi, off in pk:
            unit_of[(l, bi)] = (nm, off)
    wins = []
    for l in (1, 2, 3):
        R = MAXR[l]
        for bi in range(l):
            nm, off = unit_of[(l, bi)]
            # rhs base partition must be 0, 32, or 64
            qoff = off if off in (0, 32, 64) else 0
            for bj in range(l):
                f0 = BLOCK_OFF[(l, bi, bj)]
                for kt in range(f0 // 128, (f0 + R - 1) // 128 + 1):
                    wins.append((nm, bi, qoff, R, bj, f0, kt))
    wins.sort(key=lambda t: t[6])
    return [(nm, bi, qoff, R, bj, f0, kt, w)
            for w, (nm, bi, qoff, R, bj, f0, kt) in enumerate(wins)]


P2WINS = _phase2_windows()
NP2 = len(P2WINS)


def _build_UTP(ws):
    """[128, NP2, 128] f32 window operator: window w (for block (nm,bj) x
    k-tile kt) holds U^T entries at rows off..off+R, col p = f0 + d - kt*128
    for the in-window coefficient range."""
    out = np.zeros((128, NP2, 128), np.float32)
    for (nm, bi, qoff, R, bj, f0, kt, w) in P2WINS:
        for nm2, l, pk, n in UNITS:
            if nm2 == nm:
                F = FBLK[l]
                break
        wv = np.array([ws[l][bi * l + bj][(F - 1) // 2 - d]
                       for d in range(R)], np.float32)
        U = np.zeros((R, R), np.float32)
        for r in range(R):
            U[r, r] = wv[r]
            U[r, r + 1:] = wv[r + 1:] - wv[r:-1]
        d0 = max(0, kt * 128 - f0)
        d1 = min(R, (kt + 1) * 128 - f0)
        p0 = f0 + d0 - kt * 128
        out[qoff:qoff + R, w, p0:p0 + (d1 - d0)] = U.T[:, d0:d1]
    return out.reshape(128, -1)


def _feat_major(v, ktiles):
    out = np.zeros((128, ktiles), np.float32)
    n = v.shape[0]
    for kt in range(ktiles):
        lo, hi = kt * 128, min((kt + 1) * 128, n)
        if lo >= n:
            break
        out[:hi - lo, kt] = v[lo:hi]
    return out


# ---------------------------------------------------------------- bass build
def build_program(n_cores=N_CORES, img_per_core=IMG_PER_CORE):
    B = img_per_core
    NG = B // 2
    assert B % 2 == 0
    nc = bacc.Bacc("TRN2", target_bir_lowering=False, debug=False,
                   num_devices=n_cores)

    AX = mybir.AxisListType
    OP = mybir.AluOpType
    ACT = mybir.ActivationFunctionType

    # ---- DRAM I/O
    # x re-laid on host: [group, half, 128, img-in-pair, EPAD] bf16
    x = nc.dram_tensor("x", [NG, 2, 128, 2, EPAD], BF16, kind="ExternalInput")
    at_d = {nm: nc.dram_tensor(f"at_{nm}", [128, 2 * n], BF16,
                               kind="ExternalInput")
            for nm, l, pk, n in UNITS}
    mk_d = {nm: nc.dram_tensor(f"mk_{nm}", [128, 2 * IMAGE], F32,
                               kind="ExternalInput")
            for nm, l, pk, n in UNITS}
    ut_d = nc.dram_tensor("ut", [128, NP2 * 128], F32, kind="ExternalInput")
    wt_d, WKT = {}, {}
    for i in range(1, 6):
        kin, nout = FC_DIMS[i - 1], FC_DIMS[i]
        WKT[i] = (kin + 1 + 127) // 128
        wt_d[i] = nc.dram_tensor(f"w{i}t", [WKT[i] * 128, nout], BF16,
                                 kind="ExternalInput")
    gm_d = nc.dram_tensor("gmt", [128, 6], F32, kind="ExternalInput")
    be_d = nc.dram_tensor("bet", [128, 6], F32, kind="ExternalInput")
    id_d = nc.dram_tensor("idn", [B, B], BF16, kind="ExternalInput")
    on_d = nc.dram_tensor("ones", [1, B], BF16, kind="ExternalInput")
    out_d = nc.dram_tensor("out", [B, NUM_CLASSES], F32, kind="ExternalOutput")

    with tile.TileContext(nc) as tc:
        with tc.tile_pool(name="consts", bufs=1) as cp_pool, \
             tc.tile_pool(name="xg", bufs=8) as xg_pool, \
             tc.tile_pool(name="scr", bufs=4) as scr_pool, \
             tc.tile_pool(name="wc", bufs=6) as wc_pool, \
             tc.tile_pool(name="dram", bufs=1, space="DRAM") as dram_pool:

            # ---- constants into SBUF (sync ring; x streams on scalar ring)
            at_sb, mk_sb, q_sb = {}, {}, {}
            for nm, l, pk, n in UNITS:
                at_sb[nm] = cp_pool.tile([128, 2, n], BF16, name=f"at_{nm}_sb")
                nc.sync.dma_start(out=at_sb[nm][:], in_=at_d[nm].ap())
                mk_sb[nm] = cp_pool.tile([128, 2, l, FBLK[l]], F32,
                                         name=f"mk_{nm}_sb")
                nc.sync.dma_start(out=mk_sb[nm][:], in_=mk_d[nm].ap())
                q_sb[(nm, 0)] = cp_pool.tile([n, B // 2, l], F32,
                                             name=f"q_{nm}a")
                q_sb[(nm, 1)] = cp_pool.tile([n, B // 2, l], F32,
                                             name=f"q_{nm}b")
            ut_sb = cp_pool.tile([128, NP2, 128], F32, name="ut_sb")
            nc.sync.dma_start(out=ut_sb[:], in_=ut_d.ap())
            gm_sb = cp_pool.tile([128, 6], F32, name="gm_sb")
            be_sb = cp_pool.tile([128, 6], F32, name="be_sb")
            id_sb = cp_pool.tile([B, B], BF16, name="id_sb")
            nc.sync.dma_start(out=gm_sb[:], in_=gm_d.ap())
            nc.sync.dma_start(out=be_sb[:], in_=be_d.ap())
            nc.sync.dma_start(out=id_sb[:], in_=id_d.ap())
            junk = cp_pool.tile([128, IMAGE], BF16, name="junk")

            # ---- phase 2+3 per 16-image half: c = U @ Q at final feature
            # partitions, then BN partial stats + AllReduce
            HB = B // 2
            c_sb = {h: cp_pool.tile([128, 6, HB], F32, name=f"c_sb{h}")
                    for h in (0, 1)}
            csq = {h: cp_pool.tile([128, 6, HB], F32, name=f"csq{h}")
                   for h in (0, 1)}
            stats = {h: cp_pool.tile([128, 12], F32, name=f"stats{h}")
                     for h in (0, 1)}
            statg = {h: cp_pool.tile([128, 12], F32, name=f"statg{h}")
                     for h in (0, 1)}
            cc_in = {h: dram_pool.tile([128, 12], F32, name=f"cc_in{h}")
                     for h in (0, 1)}
            cc_out = {h: dram_pool.tile(
                [128, 12], F32, name=f"cc_out{h}",
                addr_space="Shared" if n_cores > 4 else "Local")
                for h in (0, 1)}
            kt_first = {}
            for (nm, bi, qoff, R, bj, f0, kt, w) in P2WINS:
                kt_first.setdefault(kt, w)

            def _phase2_stats(h):
                cs = c_sb[h]
                qre = {}
                for bi, off in [(1, 37), (2, 74)]:
                    qre[bi] = cp_pool.tile([37, HB, 3], F32,
                                           name=f"qre{bi}h{h}")
                    nc.sync.dma_start(
                        out=qre[bi][:],
                        in_=q_sb[("u3", h)][off:off + 37, :, :])
                with tc.tile_pool(name=f"cps{h}", bufs=2,
                                  space="PSUM") as cps_pool:
                    for (nm, bi, qoff, R, bj, f0, kt, w) in P2WINS:
                        if nm == "u3" and bi >= 1:
                            qsrc = qre[bi][0:R, :, bj]
                        else:
                            qsrc = q_sb[(nm, h)][qoff:qoff + R, :, bj]
                        pw = cps_pool.tile([128, HB], F32, tag="pw",
                                           name=f"pw{w}h{h}")
                        nc.tensor.matmul(
                            pw[:], ut_sb[qoff:qoff + R, w, :], qsrc,
                            start=True, stop=True)
                        if kt_first[kt] == w:
                            nc.scalar.copy(cs[0:128, kt, :], pw[:])
                        else:
                            nc.vector.tensor_tensor(
                                cs[0:128, kt, :], cs[0:128, kt, :], pw[:],
                                OP.add)
                nc.vector.tensor_reduce(stats[h][0:128, 0:6], cs[:], AX.X,
                                        OP.add)
                nc.vector.tensor_tensor(csq[h][:], cs[:], cs[:], OP.mult)
                nc.vector.tensor_reduce(stats[h][0:128, 6:12], csq[h][:],
                                        AX.X, OP.add)
                nc.sync.dma_start(out=cc_in[h][:], in_=stats[h][:])
                nc.gpsimd.collective_compute(
                    "AllReduce", OP.add,
                    replica_groups=[list(range(n_cores))],
                    ins=[cc_in[h][:].opt()], outs=[cc_out[h][:].opt()])
                nc.sync.dma_start(out=statg[h][:], in_=cc_out[h][:])

            # ---- phase 1: stream images, segment-reduce to Q
            with tc.tile_pool(name="zp", bufs=2, space="PSUM") as zp_pool:
                for g in range(NG):
                    xg = xg_pool.tile([128, 2, 2, EPAD // 3, 3], BF16,
                                      tag="xg")
                    for s in (0, 1):
                        nc.scalar.dma_start(
                            out=xg[0:128, s, :, :, :],
                            in_=x.ap()[g, s, 0:128, :, :])
                    for nm, l, pk, n in UNITS:
                        F = FBLK[l]
                        zp = zp_pool.tile([n, 2, l, F], F32, tag=f"z{nm}")
                        nc.tensor.matmul(
                            zp[:], at_sb[nm][0:128, 0, :],
                            xg[0:128, 0, :, 0:IMAGE, l - 1],
                            start=True, stop=False)
                        nc.tensor.matmul(
                            zp[:], at_sb[nm][0:128, 1, :],
                            xg[0:128, 1, :, 0:IMAGE, l - 1],
                            start=False, stop=True)
                        sc = scr_pool.tile([n, 2, l, F], BF16, tag=f"sc{nm}")
                        nc.vector.tensor_tensor(
                            sc[:], zp[:], mk_sb[nm][0:n, :, :, :], OP.mult)
                        hf_, gc = g // (NG // 2), g % (NG // 2)
                        if nm in SCALAR_REDUCE:
                            for j in (0, 1):
                                for bj in range(l):
                                    nc.scalar.activation(
                                        junk[0:n, 0:F], sc[0:n, j, bj, :],
                                        ACT.Copy,
                                        accum_out=q_sb[(nm, hf_)][
                                            0:n, 2 * gc + j, bj:bj + 1])
                        else:
                            nc.vector.tensor_reduce(
                                q_sb[(nm, hf_)][0:n, 2 * gc:2 * gc + 2, :],
                                sc[:], AX.X, OP.add)
                    if g == NG // 2 - 1:
                        _phase2_stats(0)
                _phase2_stats(1)

            # ---- phase 4: d = gamma * rsqrt(var + eps); c <- d*c + e
            nb = float(n_cores * B)
            statsum = cp_pool.tile([128, 12], F32, name="statsum")
            nc.vector.tensor_tensor(statsum[:], statg[0][:], statg[1][:],
                                    OP.add)
            bnd = cp_pool.tile([128, 6], F32, name="bnd")
            bne = cp_pool.tile([128, 6], F32, name="bne")
            mu = cp_pool.tile([128, 6], F32, name="mu")
            vtmp = cp_pool.tile([128, 6], F32, name="vtmp")
            nc.scalar.mul(mu[:], statsum[0:128, 0:6], 1.0 / nb)
            nc.scalar.mul(vtmp[:], statsum[0:128, 6:12], 1.0 / nb)
            nc.vector.tensor_tensor(bnd[:], mu[:], mu[:], OP.mult)
            nc.vector.tensor_tensor(vtmp[:], vtmp[:], bnd[:], OP.subtract)
            eps = cp_pool.tile([128, 1], F32, name="eps")
            nc.vector.memset(eps[:], 1e-5)
            nc.scalar.activation(vtmp[:], vtmp[:], ACT.Sqrt, bias=eps[:])
            nc.vector.reciprocal(vtmp[:], vtmp[:])
            nc.vector.tensor_tensor(bnd[:], gm_sb[:], vtmp[:], OP.mult)
            nc.vector.tensor_tensor(vtmp[:], mu[:], bnd[:], OP.mult)
            nc.vector.tensor_tensor(bne[:], be_sb[:], vtmp[:], OP.subtract)
            cn_sb = cp_pool.tile([128, 6, B], BF16, name="cn_sb")
            nc.vector.memset(cn_sb[0:128, 5, :], 0.0)
            # ones row for the fc1 bias at feature 668 = ktile 5, partition 28
            nc.sync.dma_start(out=cn_sb[28:29, 5, :], in_=on_d.ap())
            for kt in range(6):
                pmax = 128 if kt < 5 else 28
                for h in (0, 1):
                    nc.vector.tensor_scalar(
                        out=cn_sb[0:pmax, kt, h * HB:(h + 1) * HB],
                        in0=c_sb[h][0:pmax, kt, :],
                        scalar1=bnd[0:pmax, kt:kt + 1],
                        scalar2=bne[0:pmax, kt:kt + 1],
                        op0=OP.mult, op1=OP.add)

            # ---- phase 5: batch-major MLP, bf16 weights stream in chunks
            h_feat = cn_sb           # (128, KT, B) feature-major, bias ones in
            with tc.tile_pool(name="mps", bufs=1, space="PSUM") as mps_pool, \
                 tc.tile_pool(name="tps", bufs=2, space="PSUM") as tps_pool:
                for i in range(1, 6):
                    kin, nout = FC_DIMS[i - 1], FC_DIMS[i]
                    KT = WKT[i]
                    nslices = [(s, min(s + 512, nout))
                               for s in range(0, nout, 512)]
                    psums = [mps_pool.tile([B, n1 - n0], F32, tag=f"mp{si}",
                                           name=f"mp{i}_{si}")
                             for si, (n0, n1) in enumerate(nslices)]
                    for kt in range(KT):
                        wck = wc_pool.tile([128, nout], BF16, tag="wc")
                        nc.sync.dma_start(
                            out=wck[:],
                            in_=wt_d[i].ap()[128 * kt:128 * (kt + 1), :])
                        for si, (n0, n1) in enumerate(nslices):
                            nc.tensor.matmul(
                                psums[si][:],
                                h_feat[0:128, kt, :],
                                wck[0:128, n0:n1],
                                start=(kt == 0), stop=(kt == KT - 1))
                    if i == 5:
                        out_sb = cp_pool.tile([B, NUM_CLASSES], F32,
                                              name="out_sb")
                        for si, (n0, n1) in enumerate(nslices):
                            nc.scalar.copy(out_sb[0:B, n0:n1], psums[si][:])
                        break
                    h_b = cp_pool.tile([B, nout], BF16, name=f"hb{i}")
                    for si, (n0, n1) in enumerate(nslices):
                        nc.scalar.activation(h_b[0:B, n0:n1], psums[si][:],
                                             ACT.Relu)
                    # transpose back to feature-major; bias ones k-tile last
                    nkt = nout // 128
                    h_feat = cp_pool.tile([128, WKT[i + 1], B], BF16,
                                          name=f"hf{i}")
                    nc.vector.memset(h_feat[0:128, nkt, :], 0.0)
                    nc.sync.dma_start(out=h_feat[0:1, nkt, :],
                                      in_=on_d.ap())
                    for kt in range(nkt):
                        tp = tps_pool.tile([128, B], BF16, tag="tp")
                        nc.tensor.transpose(
                            tp[:], h_b[0:B, kt * 128:(kt + 1) * 128],
                            id_sb[:])
                        nc.scalar.copy(h_feat[0:128, kt, :], tp[:])
            nc.sync.dma_start(out=out_d.ap(), in_=out_sb[:])

    nc.compile()
    return nc


# ------------------------------------------------------------------- runtime
_CACHE = {}


def _get_program():
    key = (N_CORES, IMG_PER_CORE)
    if key not in _CACHE:
        _CACHE[key] = build_program(*key)
    return _CACHE[key]


def _host_consts(w1, w2, w3, bn_gamma, bn_beta, fcs):
    ws = {1: np.asarray(w1, np.float32).reshape(1, -1),
          2: np.asarray(w2, np.float32),
          3: np.asarray(w3, np.float32)}
    consts = {}
    for nm, l, pk, n in UNITS:
        consts[f"at_{nm}"] = _unit_A2(nm).reshape(128, -1)
        consts[f"mk_{nm}"] = _unit_M(nm)
    consts["ut"] = _build_UTP(ws)
    for i in range(1, 6):
        w, b = fcs[i - 1]
        kin, nout = FC_DIMS[i - 1], FC_DIMS[i]
        kt = (kin + 1 + 127) // 128
        wt = np.zeros((kt * 128, nout), np.float32)
        wt[0:kin] = np.asarray(w, np.float32).T
        wt[kin] = np.asarray(b, np.float32)
        consts[f"w{i}t"] = np.ascontiguousarray(wt.astype(BF16NP))
    consts["gmt"] = _feat_major(np.asarray(bn_gamma, np.float32), 6)
    consts["bet"] = _feat_major(np.asarray(bn_beta, np.float32), 6)
    consts["idn"] = np.eye(IMG_PER_CORE, dtype=BF16NP)
    consts["ones"] = np.ones((1, IMG_PER_CORE), BF16NP)
    return consts


def _layout_x(xs):
    """[32, 222, 222, 3] f32 -> [16, 2, 128, 2, EPAD] bf16 (rows 0..127 and
    94..221 per image, elems padded 666 -> EPAD)."""
    B = xs.shape[0]
    xi = xs.reshape(B, IMAGE, IMAGE * 3).astype(BF16NP)
    out = np.zeros((B // 2, 2, 128, 2, EPAD), BF16NP)
    # [B, rows, 666] -> groups of 2 imgs: [NG, 2, rows, 666]
    xg = xi.reshape(B // 2, 2, IMAGE, IMAGE * 3)
    out[:, 0, :, :, 0:IMAGE * 3] = xg[:, :, 0:128].transpose(0, 2, 1, 3)
    out[:, 1, :, :, 0:IMAGE * 3] = xg[:, :, H2_OFF:IMAGE].transpose(0, 2, 1, 3)
    return np.ascontiguousarray(out)


def kernel(x, w1, w2, w3, bn_gamma, bn_beta,
           fc1_w, fc1_b, fc2_w, fc2_b, fc3_w, fc3_b, fc4_w, fc4_b,
           fc5_w, fc5_b):
    from concourse.bass_utils import run_bass_kernel_spmd

    nc = _get_program()
    consts = _host_consts(
        w1, w2, w3, bn_gamma, bn_beta,
        [(fc1_w, fc1_b), (fc2_w, fc2_b), (fc3_w, fc3_b), (fc4_w, fc4_b),
         (fc5_w, fc5_b)])
    x = np.asarray(x, np.float32)
    in_maps = []
    for s in range(N_CORES):
        m = dict(consts)
        m["x"] = _layout_x(x[s * IMG_PER_CORE:(s + 1) * IMG_PER_CORE])
        in_maps.append(m)

    trace = bool(int(os.environ.get("BASSDFT_TRACE", "0")))
    if trace:
        _install_ntff_hook()
    res = run_bass_kernel_spmd(nc, in_maps, core_ids=list(range(N_CORES)),
                               trace=trace)
    if trace:
        kernel.last_exec_time_ns = res.exec_time_ns
        kernel.last_results = res
    return np.concatenate([res.results[s]["out"] for s in range(N_CORES)],
                          axis=0)


def _install_ntff_hook():
    """Register the axon NTFF profiling hook (antenv.axon_hooks is absent in
    this image) and disable the share-bucket artifact upload."""
    try:
        from antenv import axon_hooks  # noqa: F401
        return
    except ImportError:
        pass
    try:
        from trn_agent_boot.trn_boot import _ntff_profile_via_ctypes
    except ImportError:
        return
    import antenv
    import concourse.bass_utils as bu
    mod = types.ModuleType("antenv.axon_hooks")
    hook = [_ntff_profile_via_ctypes("/opt/axon/libaxon_pjrt.so")]
    mod.get_axon_ntff_profile_hook = lambda: hook[0]
    mod.set_axon_ntff_profile_hook = lambda h: hook.__setitem__(0, h)
    sys.modules["antenv.axon_hooks"] = mod
    antenv.axon_hooks = mod
    bu.upload_artifacts = lambda tmpdir: tmpdir
